# revision 8
# baseline (speedup 1.0000x reference)
"""Multi-head attention (nn_MultiHeadAttention) on 8 Trainium2 NeuronCores.

Head-parallel sharding: each of the 8 cores owns 2 of the 16 heads and the
matching rows of Wo. Each core computes its heads' full attention plus its
partial output projection; the host sums the 8 partials and adds bo.

Math notes (matching the reference):
  attn = softmax(scores) / (2*DK)   with 2*DK == 128
  The softmax denominator is produced inside the attn@V matmul by an extra
  lhsT column holding the constant 128.0, so psum row 64 = 128*sum(exp).
  V-projection bias bv is folded in by augmenting the contraction with a
  K=1 matmul of a constant-one row against [bv | 128.0], which also makes
  bias handling exact: (numer + bv*denom)/denom = attn_out + bv.
"""

from contextlib import ExitStack

import numpy as np
import ml_dtypes

import concourse.bass as bass
import concourse.tile as tile
from concourse import bacc
from concourse import mybir

F32 = mybir.dt.float32
F32R = mybir.dt.float32r
BF16 = mybir.dt.bfloat16
EXP = mybir.ActivationFunctionType.Exp

B, S, D, NH, DK, DV = 2, 2048, 1024, 16, 64, 64
NCORES = 8
HPC = NH // NCORES  # heads per core == 2


def build_nc(b=B, s=S, d=D, dk=DK):
    """Build the per-core Bass program (identical on all 8 cores)."""
    nc = bacc.Bacc("TRN2", target_bir_lowering=False, debug=False)

    sq_t = min(512, s)          # sq tile (matmul free dim)
    n_sq = s // sq_t
    n_sk = s // 128             # sk tiles of 128
    n_ch = d // 128             # contraction chunks of 128
    n_dh = (d + 511) // 512     # output d halves
    d_h = min(512, d)

    qT_d = nc.dram_tensor("qT", [b, d, s], F32R, kind="ExternalInput").ap()
    kT_d = nc.dram_tensor("kT", [b, d, s], F32R, kind="ExternalInput").ap()
    vT_d = nc.dram_tensor("vT", [b, d, s], BF16, kind="ExternalInput").ap()
    wq_d = nc.dram_tensor("wq", [d, 128], F32R, kind="ExternalInput").ap()
    wk_d = nc.dram_tensor("wk", [d, 128], F32R, kind="ExternalInput").ap()
    wv_d = nc.dram_tensor("wv", [d, 130], BF16, kind="ExternalInput").ap()
    bqk_d = nc.dram_tensor("bqk", [128, 2], F32, kind="ExternalInput").ap()
    bvr_d = nc.dram_tensor("bv_row", [1, 130], BF16, kind="ExternalInput").ap()
    woa_d = nc.dram_tensor("wo_a", [64, d], BF16, kind="ExternalInput").ap()
    ones16_d = nc.dram_tensor("ones16", [1, 128], BF16, kind="ExternalInput").ap()
    onesr_d = nc.dram_tensor("onesr", [128, 64], F32R, kind="ExternalInput").ap()
    wob_d = nc.dram_tensor("wo_b", [64, d], BF16, kind="ExternalInput").ap()
    out_d = nc.dram_tensor("out", [b, s, d], F32, kind="ExternalOutput").ap()

    with tile.TileContext(nc) as tc, ExitStack() as ctx:
        consts = ctx.enter_context(tc.tile_pool(name="consts", bufs=1))
        qk_stream = ctx.enter_context(tc.tile_pool(name="qk_stream", bufs=3))
        vt_stream = ctx.enter_context(tc.tile_pool(name="vt_stream", bufs=3))
        qkt_pool = ctx.enter_context(tc.tile_pool(name="qkt", bufs=2))
        vh_pool = ctx.enter_context(tc.tile_pool(name="vh", bufs=2))
        exp_pool = ctx.enter_context(tc.tile_pool(name="expp", bufs=4))
        cat_pool = ctx.enter_context(tc.tile_pool(name="cat", bufs=2))
        recip_pool = ctx.enter_context(tc.tile_pool(name="recip", bufs=2))
        out_pool = ctx.enter_context(tc.tile_pool(name="outp", bufs=3))
        ps = ctx.enter_context(tc.tile_pool(name="ps", bufs=8, space="PSUM"))

        # --- constants ---
        wq_sb = consts.tile([128, n_ch, 128], F32R, tag="wq")
        wk_sb = consts.tile([128, n_ch, 128], F32R, tag="wk")
        wv_sb = consts.tile([128, n_ch, 130], BF16, tag="wv")
        for c in range(n_ch):
            nc.sync.dma_start(wq_sb[:, c, :], wq_d[c * 128:(c + 1) * 128, :])
            nc.sync.dma_start(wk_sb[:, c, :], wk_d[c * 128:(c + 1) * 128, :])
            nc.sync.dma_start(wv_sb[:, c, :], wv_d[c * 128:(c + 1) * 128, :])
        bqk_sb = consts.tile([128, 2], F32, tag="bqk")
        nc.sync.dma_start(bqk_sb[:], bqk_d[:])
        bvr_sb = consts.tile([1, 130], BF16, tag="bvr")
        nc.sync.dma_start(bvr_sb[:], bvr_d[:])
        woa_sb = consts.tile([64, d], BF16, tag="woa")
        nc.sync.dma_start(woa_sb[:], woa_d[:])
        wob_sb = consts.tile([64, d], BF16, tag="wob")
        nc.sync.dma_start(wob_sb[:], wob_d[:])
        ones_bf = consts.tile([1, 128], BF16, tag="ones_bf")
        nc.sync.dma_start(ones_bf[:], ones16_d[:])
        ones_fr = consts.tile([128, 64], F32R, tag="ones_fr")
        nc.sync.dma_start(ones_fr[:], onesr_d[:])

        for bi in range(b):
            # ---- Phase P1: q/k head projections -> qhT/khT [128, s] (f32r)
            qhT = qkt_pool.tile([128, s], F32R, tag="qhT")
            khT = qkt_pool.tile([128, s], F32R, tag="khT")
            psq = [ps.tile([128, sq_t], F32, tag="ps", name=f"psq{i}") for i in range(n_sq)]
            psk = [ps.tile([128, sq_t], F32, tag="ps", name=f"psk{i}") for i in range(n_sq)]
            for c in range(n_ch):
                qt_c = qk_stream.tile([128, s], F32R, tag="qt")
                nc.sync.dma_start(qt_c[:], qT_d[bi, c * 128:(c + 1) * 128, :])
                kt_c = qk_stream.tile([128, s], F32R, tag="kt")
                nc.sync.dma_start(kt_c[:], kT_d[bi, c * 128:(c + 1) * 128, :])
                for st in range(n_sq):
                    ssl = bass.ts(st, sq_t)
                    nc.tensor.matmul(psq[st][:], wq_sb[:, c, :], qt_c[:, ssl],
                                     start=(c == 0), stop=(c == n_ch - 1))
                    nc.tensor.matmul(psk[st][:], wk_sb[:, c, :], kt_c[:, ssl],
                                     start=(c == 0), stop=(c == n_ch - 1))
            for st in range(n_sq):
                ssl = bass.ts(st, sq_t)
                with nc.allow_low_precision(reason="f32r == f32 bits"):
                    nc.vector.tensor_scalar_add(qhT[:, ssl], psq[st][:],
                                                bqk_sb[:, 0:1])
                    nc.vector.tensor_scalar_add(khT[:, ssl], psk[st][:],
                                                bqk_sb[:, 1:2])

            # ---- Phase P2: v projection -> vh [128(t), n_sk, 130] (bf16)
            # per t-tile psum cols: [vhA+bvA (64) | 128.0 | vhB+bvB (64) | 128.0]
            vh = vh_pool.tile([128, n_sk, 130], BF16, tag="vh")
            for tt in range(n_sk):
                vt_c = vt_stream.tile([128, n_ch, 128], BF16, tag="vt")
                nc.sync.dma_start(
                    vt_c[:],
                    vT_d[bi].rearrange("(c p) s -> p c s", p=128)[
                        :, :, bass.ts(tt, 128)],
                )
                psv = ps.tile([128, 512], F32, tag="ps")
                for c in range(n_ch):
                    nc.tensor.matmul(psv[:, 0:130], vt_c[:, c, :], wv_sb[:, c, :],
                                     start=(c == 0), stop=False)
                nc.tensor.matmul(psv[:, 0:130], ones_bf[:], bvr_sb[:],
                                 start=False, stop=True)
                nc.any.tensor_copy(vh[:, tt, 0:65], psv[:, 0:65])
                nc.any.tensor_copy(vh[:, tt, 65:130], psv[:, 65:130])

            # ---- Phase A: attention for both heads
            catA = cat_pool.tile([64, s], BF16, tag="catA")
            catB = cat_pool.tile([64, s], BF16, tag="catB")
            for sq in range(n_sq):
                ssl = bass.ts(sq, sq_t)
                nA = ps.tile([128, sq_t], F32, tag="ps")
                nB = ps.tile([128, sq_t], F32, tag="ps")
                for k in range(n_sk):
                    ksl = bass.ts(k, 128)
                    sA = ps.tile([128, sq_t], F32, tag="ps")
                    sB = ps.tile([128, sq_t], F32, tag="ps")
                    nc.tensor.matmul(sA[:], khT[0:64, ksl], qhT[0:64, ssl],
                                     start=True, stop=True, tile_position=(0, 0))
                    nc.tensor.matmul(sB[:], khT[64:128, ksl], qhT[64:128, ssl],
                                     start=True, stop=True, tile_position=(64, 0))
                    eA = exp_pool.tile([128, sq_t], BF16, tag="eA")
                    nc.scalar.activation(eA[:], sA[:], EXP)
                    eB = exp_pool.tile([128, sq_t], BF16, tag="eB")
                    nc.scalar.activation(eB[:], sB[:], EXP)
                    nc.tensor.matmul(nA[0:65, :], vh[:, k, 0:65], eA[:],
                                     start=(k == 0), stop=(k == n_sk - 1))
                    nc.tensor.matmul(nB[0:65, :], vh[:, k, 65:130], eB[:],
                                     start=(k == 0), stop=(k == n_sk - 1))
                rec = recip_pool.tile([128, 2 * sq_t], F32R, tag="rec")
                with nc.allow_low_precision(reason="f32r == f32 bits"):
                    nc.vector.reciprocal(rec[64:65, 0:sq_t], nA[64:65, :])
                    nc.vector.reciprocal(rec[64:65, sq_t:2 * sq_t], nB[64:65, :])
                bcA = ps.tile([128, sq_t], F32, tag="ps")
                bcB = ps.tile([128, sq_t], F32, tag="ps")
                nc.tensor.matmul(bcA[0:64, :], ones_fr[64:65, :],
                                 rec[64:65, 0:sq_t], start=True, stop=True)
                nc.tensor.matmul(bcB[0:64, :], ones_fr[64:65, :],
                                 rec[64:65, sq_t:2 * sq_t], start=True, stop=True)
                # DVE can only read one PSUM operand per op: bounce the
                # broadcast reciprocal through SBUF first.
                rb = recip_pool.tile([64, 2 * sq_t], F32, tag="rb")
                nc.any.tensor_copy(rb[:, 0:sq_t], bcA[0:64, :])
                nc.any.tensor_copy(rb[:, sq_t:2 * sq_t], bcB[0:64, :])
                nc.vector.tensor_mul(catA[:, ssl], nA[0:64, :], rb[:, 0:sq_t])
                nc.vector.tensor_mul(catB[:, ssl], nB[0:64, :],
                                     rb[:, sq_t:2 * sq_t])

            # ---- Phase O: output projection (per-head K=64 accumulate)
            for ot in range(s // 128):
                osl = bass.ts(ot, 128)
                o_sb = out_pool.tile([128, d], F32, tag="o")
                for dh in range(n_dh):
                    dsl = bass.ts(dh, d_h)
                    po = ps.tile([128, d_h], F32, tag="ps")
                    nc.tensor.matmul(po[:], catA[:, osl], woa_sb[:, dsl],
                                     start=True, stop=False)
                    nc.tensor.matmul(po[:], catB[:, osl], wob_sb[:, dsl],
                                     start=False, stop=True)
                    nc.any.tensor_copy(o_sb[:, dsl], po[:])
                nc.sync.dma_start(out_d[bi, ot * 128:(ot + 1) * 128, :], o_sb[:])

    nc.compile()
    return nc


def make_core_inputs(Q, K, V, Wq, bq, Wk, bk, Wv, bv, Wo):
    """Host-side prep: transposes, casts, per-core weight packing."""
    bf = ml_dtypes.bfloat16
    QT = np.ascontiguousarray(np.transpose(np.asarray(Q, np.float32), (0, 2, 1)))
    KT = np.ascontiguousarray(np.transpose(np.asarray(K, np.float32), (0, 2, 1)))
    VT = np.ascontiguousarray(
        np.transpose(np.asarray(V, np.float32), (0, 2, 1))).astype(bf)
    d = QT.shape[1]

    in_maps = []
    for c in range(NCORES):
        hA, hB = HPC * c, HPC * c + 1
        wq = np.concatenate([Wq[hA], Wq[hB]], axis=1).astype(np.float32)
        wk = np.concatenate([Wk[hA], Wk[hB]], axis=1).astype(np.float32)
        wv = np.zeros((d, 130), np.float32)
        wv[:, 0:64] = Wv[hA]
        wv[:, 65:129] = Wv[hB]
        bvr = np.zeros((1, 130), np.float32)
        bvr[0, 0:64] = bv[hA]
        bvr[0, 64] = 128.0
        bvr[0, 65:129] = bv[hB]
        bvr[0, 129] = 128.0
        bqk = np.stack(
            [np.concatenate([bq[hA], bq[hB]]), np.concatenate([bk[hA], bk[hB]])],
            axis=1).astype(np.float32)
        in_maps.append({
            "qT": QT, "kT": KT, "vT": VT,
            "wq": wq, "wk": wk, "wv": wv.astype(bf),
            "bqk": bqk, "bv_row": bvr.astype(bf),
            "ones16": np.ones((1, 128), np.float32).astype(bf),
            "onesr": np.ones((128, 64), np.float32),
            "wo_a": np.asarray(Wo[64 * hA:64 * hA + 64], np.float32).astype(bf),
            "wo_b": np.asarray(Wo[64 * hB:64 * hB + 64], np.float32).astype(bf),
        })
    return in_maps


_NC_CACHE = {}


def _get_nc():
    if "nc" not in _NC_CACHE:
        _NC_CACHE["nc"] = build_nc()
    return _NC_CACHE["nc"]


def _install_ntff_hook_shim():
    """The agent image's antenv lacks axon_hooks; recreate the tiny
    get/set registry and register the ctypes NTFF profiler so trace=True
    can report HW exec time."""
    import sys
    import types
    if "antenv.axon_hooks" in sys.modules:
        return
    hook = None
    try:
        from trn_agent_boot.trn_boot import _ntff_profile_via_ctypes
        hook = _ntff_profile_via_ctypes("/opt/axon/libaxon_pjrt.so")
    except Exception:
        hook = None
    mod = types.ModuleType("antenv.axon_hooks")
    mod._hook = hook
    mod.get_axon_ntff_profile_hook = lambda: mod._hook
    mod.set_axon_ntff_profile_hook = lambda h: setattr(mod, "_hook", h)
    sys.modules["antenv.axon_hooks"] = mod


def kernel(Q, K, V, Wq, bq, Wk, bk, Wv, bv, Wo, bo, _trace=False):
    from concourse.bass_utils import run_bass_kernel_spmd

    if _trace:
        _install_ntff_hook_shim()

    nc = _get_nc()
    in_maps = make_core_inputs(Q, K, V, Wq, bq, Wk, bk, Wv, bv, Wo)
    res = run_bass_kernel_spmd(nc, in_maps, list(range(NCORES)), trace=_trace)
    out = np.zeros((B, S, D), np.float32)
    for r in res.results:
        out += np.asarray(r["out"], np.float32)
    out += np.asarray(bo, np.float32)[None, None, :]
    if _trace:
        return out, res
    return out


# revision 11
# speedup vs baseline: 1.4561x; 1.4561x over previous
"""Multi-head attention (nn_MultiHeadAttention) on 8 Trainium2 NeuronCores.

Head-parallel sharding: each of the 8 cores owns 2 of the 16 heads and the
matching rows of Wo. Each core computes its heads' full attention plus its
partial output projection; the host sums the 8 partials and adds bo.

Math notes (matching the reference):
  attn = softmax(scores) / (2*DK)   with 2*DK == 128
  The softmax denominator is produced inside the attn@V matmul by an extra
  lhsT column holding the constant 128.0, so psum row 64 = 128*sum(exp).
  V-projection bias bv is folded in by augmenting the contraction with a
  K=1 matmul of a constant-one row against [bv | 128.0], which also makes
  bias handling exact: (numer + bv*denom)/denom = attn_out + bv.
"""

from contextlib import ExitStack

import numpy as np
import ml_dtypes

import concourse.bass as bass
import concourse.tile as tile
from concourse import bacc
from concourse import mybir

F32 = mybir.dt.float32
F32R = mybir.dt.float32r
BF16 = mybir.dt.bfloat16
EXP = mybir.ActivationFunctionType.Exp

B, S, D, NH, DK, DV = 2, 2048, 1024, 16, 64, 64
NCORES = 8
HPC = NH // NCORES  # heads per core == 2


def build_nc(b=B, s=S, d=D, dk=DK):
    """Build the per-core Bass program (identical on all 8 cores)."""
    nc = bacc.Bacc("TRN2", target_bir_lowering=False, debug=False)

    sq_t = min(512, s)          # sq tile (matmul free dim)
    n_sq = s // sq_t
    n_sk = s // 128             # sk tiles of 128
    n_ch = d // 128             # contraction chunks of 128
    n_dh = (d + 511) // 512     # output d halves
    d_h = min(512, d)

    qT_d = nc.dram_tensor("qT", [b, d, s], BF16, kind="ExternalInput").ap()
    kT_d = nc.dram_tensor("kT", [b, d, s], BF16, kind="ExternalInput").ap()
    vT_d = nc.dram_tensor("vT", [b, d, s], BF16, kind="ExternalInput").ap()
    wq_d = nc.dram_tensor("wq", [d, 128], BF16, kind="ExternalInput").ap()
    wk_d = nc.dram_tensor("wk", [d, 128], BF16, kind="ExternalInput").ap()
    wv_d = nc.dram_tensor("wv", [d, 130], BF16, kind="ExternalInput").ap()
    bqk_d = nc.dram_tensor("bqk", [128, 2], F32, kind="ExternalInput").ap()
    bvr_d = nc.dram_tensor("bv_row", [1, 130], BF16, kind="ExternalInput").ap()
    woa_d = nc.dram_tensor("wo_a", [64, d], BF16, kind="ExternalInput").ap()
    ones16_d = nc.dram_tensor("ones16", [1, 128], BF16, kind="ExternalInput").ap()
    onesr_d = nc.dram_tensor("onesr", [128, 64], F32R, kind="ExternalInput").ap()
    wob_d = nc.dram_tensor("wo_b", [64, d], BF16, kind="ExternalInput").ap()
    out_d = nc.dram_tensor("out", [b, s, d], F32, kind="ExternalOutput").ap()

    with tile.TileContext(nc) as tc, ExitStack() as ctx:
        consts = ctx.enter_context(tc.tile_pool(name="consts", bufs=1))
        qk_stream = ctx.enter_context(tc.tile_pool(name="qk_stream", bufs=8))
        vt_stream = ctx.enter_context(tc.tile_pool(name="vt_stream", bufs=3))
        qkt_pool = ctx.enter_context(tc.tile_pool(name="qkt", bufs=2))
        vh_pool = ctx.enter_context(tc.tile_pool(name="vh", bufs=2))
        exp_pool = ctx.enter_context(tc.tile_pool(name="expp", bufs=4))
        cat_pool = ctx.enter_context(tc.tile_pool(name="cat", bufs=2))
        recip_pool = ctx.enter_context(tc.tile_pool(name="recip", bufs=2))
        out_pool = ctx.enter_context(tc.tile_pool(name="outp", bufs=3))
        ps = ctx.enter_context(tc.tile_pool(name="ps", bufs=4, space="PSUM"))

        # --- constants ---
        wq_sb = consts.tile([128, n_ch, 128], BF16, tag="wq")
        wk_sb = consts.tile([128, n_ch, 128], BF16, tag="wk")
        wv_sb = consts.tile([128, n_ch, 130], BF16, tag="wv")
        for c in range(n_ch):
            nc.sync.dma_start(wq_sb[:, c, :], wq_d[c * 128:(c + 1) * 128, :])
            nc.sync.dma_start(wk_sb[:, c, :], wk_d[c * 128:(c + 1) * 128, :])
            nc.sync.dma_start(wv_sb[:, c, :], wv_d[c * 128:(c + 1) * 128, :])
        bqk_sb = consts.tile([128, 2], F32, tag="bqk")
        nc.sync.dma_start(bqk_sb[:], bqk_d[:])
        bvr_sb = consts.tile([1, 130], BF16, tag="bvr")
        nc.sync.dma_start(bvr_sb[:], bvr_d[:])
        woa_sb = consts.tile([64, d], BF16, tag="woa")
        nc.sync.dma_start(woa_sb[:], woa_d[:])
        wob_sb = consts.tile([64, d], BF16, tag="wob")
        nc.sync.dma_start(wob_sb[:], wob_d[:])
        ones_bf = consts.tile([1, 128], BF16, tag="ones_bf")
        nc.sync.dma_start(ones_bf[:], ones16_d[:])
        ones_fr = consts.tile([128, 64], F32R, tag="ones_fr")
        nc.sync.dma_start(ones_fr[:], onesr_d[:])

        for bi in range(b):
            # ---- Phase P1: q/k head projections -> qhT/khT [128, s] (f32r)
            qhT = qkt_pool.tile([128, s], BF16, tag="qhT")
            khT = qkt_pool.tile([128, s], BF16, tag="khT")
            qt_cs, kt_cs = [], []
            for c in range(n_ch):
                qt_c = qk_stream.tile([128, s], BF16, tag="qt", name=f"qt{c}")
                nc.sync.dma_start(qt_c[:], qT_d[bi, c * 128:(c + 1) * 128, :])
                qt_cs.append(qt_c)
                kt_c = qk_stream.tile([128, s], BF16, tag="kt", name=f"kt{c}")
                nc.sync.dma_start(kt_c[:], kT_d[bi, c * 128:(c + 1) * 128, :])
                kt_cs.append(kt_c)
            # two s-tile halves so q+k projections fit in 4 psum banks
            for half in range(0, n_sq, 2):
                sts = [st for st in (half, half + 1) if st < n_sq]
                pq = {st: ps.tile([128, sq_t], F32, tag="ps", name=f"psq{st}")
                      for st in sts}
                pk = {st: ps.tile([128, sq_t], F32, tag="ps", name=f"psk{st}")
                      for st in sts}
                for c in range(n_ch):
                    for st in sts:
                        ssl = bass.ts(st, sq_t)
                        nc.tensor.matmul(pq[st][:], wq_sb[:, c, :],
                                         qt_cs[c][:, ssl],
                                         start=(c == 0), stop=(c == n_ch - 1))
                        nc.tensor.matmul(pk[st][:], wk_sb[:, c, :],
                                         kt_cs[c][:, ssl],
                                         start=(c == 0), stop=(c == n_ch - 1))
                for st in sts:
                    ssl = bass.ts(st, sq_t)
                    with nc.allow_low_precision(reason="f32r == f32 bits"):
                        nc.vector.tensor_scalar_add(qhT[:, ssl], pq[st][:],
                                                    bqk_sb[:, 0:1])
                        nc.vector.tensor_scalar_add(khT[:, ssl], pk[st][:],
                                                    bqk_sb[:, 1:2])

            # ---- Phase P2: v projection -> vh [128(t), n_sk, 130] (bf16)
            # per t-tile psum cols: [vhA+bvA (64) | 128.0 | vhB+bvB (64) | 128.0]
            vh = vh_pool.tile([128, n_sk, 130], BF16, tag="vh")
            for tt in range(n_sk):
                vt_c = vt_stream.tile([128, n_ch, 128], BF16, tag="vt")
                nc.sync.dma_start(
                    vt_c[:],
                    vT_d[bi].rearrange("(c p) s -> p c s", p=128)[
                        :, :, bass.ts(tt, 128)],
                )
                psv = ps.tile([128, 512], F32, tag="ps")
                for c in range(n_ch):
                    nc.tensor.matmul(psv[:, 0:130], vt_c[:, c, :], wv_sb[:, c, :],
                                     start=(c == 0), stop=False)
                nc.tensor.matmul(psv[:, 0:130], ones_bf[:], bvr_sb[:],
                                 start=False, stop=True)
                nc.any.tensor_copy(vh[:, tt, 0:65], psv[:, 0:65])
                nc.any.tensor_copy(vh[:, tt, 65:130], psv[:, 65:130])

            # ---- Phase A: attention for both heads
            catA = cat_pool.tile([64, s], BF16, tag="catA")
            catB = cat_pool.tile([64, s], BF16, tag="catB")
            for sq in range(n_sq):
                ssl = bass.ts(sq, sq_t)
                nA = ps.tile([128, sq_t], F32, tag="ps")
                nB = ps.tile([128, sq_t], F32, tag="ps")
                for k in range(n_sk):
                    ksl = bass.ts(k, 128)
                    # both heads' scoresT into one 2-bank psum -> one exp
                    sAB = ps.tile([128, 2 * sq_t], F32, tag="ps2", bufs=2)
                    nc.tensor.matmul(sAB[:, 0:sq_t], khT[0:64, ksl],
                                     qhT[0:64, ssl],
                                     start=True, stop=True, tile_position=(0, 0))
                    nc.tensor.matmul(sAB[:, sq_t:2 * sq_t], khT[64:128, ksl],
                                     qhT[64:128, ssl],
                                     start=True, stop=True, tile_position=(64, 0))
                    eAB = exp_pool.tile([128, 2 * sq_t], BF16, tag="eAB")
                    nc.scalar.activation(eAB[:], sAB[:], EXP)
                    nc.tensor.matmul(nA[0:65, :], vh[:, k, 0:65], eAB[:, 0:sq_t],
                                     start=(k == 0), stop=(k == n_sk - 1))
                    nc.tensor.matmul(nB[0:65, :], vh[:, k, 65:130],
                                     eAB[:, sq_t:2 * sq_t],
                                     start=(k == 0), stop=(k == n_sk - 1))
                # free nA/nB quickly: copy numerators + denominators out of
                # PSUM before the slow reciprocal runs.
                numAB = recip_pool.tile([64, 2 * sq_t], F32, tag="numAB")
                nc.any.tensor_copy(numAB[:, 0:sq_t], nA[0:64, :])
                nc.any.tensor_copy(numAB[:, sq_t:2 * sq_t], nB[0:64, :])
                rec = recip_pool.tile([128, 4 * sq_t], F32R, tag="rec")
                nc.vector.tensor_copy(rec[64:65, 0:sq_t], nA[64:65, :])
                nc.vector.tensor_copy(rec[64:65, sq_t:2 * sq_t], nB[64:65, :])
                with nc.allow_low_precision(reason="f32r == f32 bits"):
                    nc.vector.reciprocal(rec[64:65, 2 * sq_t:3 * sq_t],
                                         rec[64:65, 0:sq_t])
                    nc.vector.reciprocal(rec[64:65, 3 * sq_t:4 * sq_t],
                                         rec[64:65, sq_t:2 * sq_t])
                bcA = ps.tile([128, sq_t], F32, tag="ps")
                bcB = ps.tile([128, sq_t], F32, tag="ps")
                nc.tensor.matmul(bcA[0:64, :], ones_fr[64:65, :],
                                 rec[64:65, 2 * sq_t:3 * sq_t],
                                 start=True, stop=True)
                nc.tensor.matmul(bcB[0:64, :], ones_fr[64:65, :],
                                 rec[64:65, 3 * sq_t:4 * sq_t],
                                 start=True, stop=True)
                nc.vector.tensor_mul(catA[:, ssl], bcA[0:64, :],
                                     numAB[:, 0:sq_t])
                nc.vector.tensor_mul(catB[:, ssl], bcB[0:64, :],
                                     numAB[:, sq_t:2 * sq_t])

            # ---- Phase O: output projection (per-head K=64 accumulate)
            for ot in range(s // 128):
                osl = bass.ts(ot, 128)
                o_sb = out_pool.tile([128, d], F32, tag="o")
                for dh in range(n_dh):
                    dsl = bass.ts(dh, d_h)
                    po = ps.tile([128, d_h], F32, tag="ps")
                    nc.tensor.matmul(po[:], catA[:, osl], woa_sb[:, dsl],
                                     start=True, stop=False)
                    nc.tensor.matmul(po[:], catB[:, osl], wob_sb[:, dsl],
                                     start=False, stop=True)
                    nc.any.tensor_copy(o_sb[:, dsl], po[:])
                nc.sync.dma_start(out_d[bi, ot * 128:(ot + 1) * 128, :], o_sb[:])

    nc.compile()
    return nc


def make_core_inputs(Q, K, V, Wq, bq, Wk, bk, Wv, bv, Wo):
    """Host-side prep: transposes, casts, per-core weight packing."""
    bf = ml_dtypes.bfloat16
    QT = np.ascontiguousarray(
        np.transpose(np.asarray(Q, np.float32), (0, 2, 1))).astype(bf)
    KT = np.ascontiguousarray(
        np.transpose(np.asarray(K, np.float32), (0, 2, 1))).astype(bf)
    VT = np.ascontiguousarray(
        np.transpose(np.asarray(V, np.float32), (0, 2, 1))).astype(bf)
    d = QT.shape[1]

    in_maps = []
    for c in range(NCORES):
        hA, hB = HPC * c, HPC * c + 1
        wq = np.concatenate([Wq[hA], Wq[hB]], axis=1).astype(np.float32).astype(bf)
        wk = np.concatenate([Wk[hA], Wk[hB]], axis=1).astype(np.float32).astype(bf)
        wv = np.zeros((d, 130), np.float32)
        wv[:, 0:64] = Wv[hA]
        wv[:, 65:129] = Wv[hB]
        bvr = np.zeros((1, 130), np.float32)
        bvr[0, 0:64] = bv[hA]
        bvr[0, 64] = 128.0
        bvr[0, 65:129] = bv[hB]
        bvr[0, 129] = 128.0
        bqk = np.stack(
            [np.concatenate([bq[hA], bq[hB]]), np.concatenate([bk[hA], bk[hB]])],
            axis=1).astype(np.float32)
        in_maps.append({
            "qT": QT, "kT": KT, "vT": VT,
            "wq": wq, "wk": wk, "wv": wv.astype(bf),
            "bqk": bqk, "bv_row": bvr.astype(bf),
            "ones16": np.ones((1, 128), np.float32).astype(bf),
            "onesr": np.ones((128, 64), np.float32),
            "wo_a": np.asarray(Wo[64 * hA:64 * hA + 64], np.float32).astype(bf),
            "wo_b": np.asarray(Wo[64 * hB:64 * hB + 64], np.float32).astype(bf),
        })
    return in_maps


_NC_CACHE = {}


def _get_nc():
    if "nc" not in _NC_CACHE:
        _NC_CACHE["nc"] = build_nc()
    return _NC_CACHE["nc"]


def _install_ntff_hook_shim():
    """The agent image's antenv lacks axon_hooks; recreate the tiny
    get/set registry and register the ctypes NTFF profiler so trace=True
    can report HW exec time."""
    import sys
    import types
    if "antenv.axon_hooks" in sys.modules:
        return
    hook = None
    try:
        from trn_agent_boot.trn_boot import _ntff_profile_via_ctypes
        hook = _ntff_profile_via_ctypes("/opt/axon/libaxon_pjrt.so")
    except Exception:
        hook = None
    mod = types.ModuleType("antenv.axon_hooks")
    mod._hook = hook
    mod.get_axon_ntff_profile_hook = lambda: mod._hook
    mod.set_axon_ntff_profile_hook = lambda h: setattr(mod, "_hook", h)
    sys.modules["antenv.axon_hooks"] = mod


def kernel(Q, K, V, Wq, bq, Wk, bk, Wv, bv, Wo, bo, _trace=False):
    from concourse.bass_utils import run_bass_kernel_spmd

    if _trace:
        _install_ntff_hook_shim()

    nc = _get_nc()
    in_maps = make_core_inputs(Q, K, V, Wq, bq, Wk, bk, Wv, bv, Wo)
    res = run_bass_kernel_spmd(nc, in_maps, list(range(NCORES)), trace=_trace)
    out = np.zeros((B, S, D), np.float32)
    for r in res.results:
        out += np.asarray(r["out"], np.float32)
    out += np.asarray(bo, np.float32)[None, None, :]
    if _trace:
        return out, res
    return out


# revision 12
# speedup vs baseline: 1.5269x; 1.0486x over previous
"""Multi-head attention (nn_MultiHeadAttention) on 8 Trainium2 NeuronCores.

Head-parallel sharding: each of the 8 cores owns 2 of the 16 heads and the
matching rows of Wo. Each core computes its heads' full attention plus its
partial output projection; the host sums the 8 partials and adds bo.

Math notes (matching the reference):
  attn = softmax(scores) / (2*DK)   with 2*DK == 128
  The softmax denominator is produced inside the attn@V matmul by an extra
  lhsT column holding the constant 128.0, so psum row 64 = 128*sum(exp).
  V-projection bias bv is folded in by augmenting the contraction with a
  K=1 matmul of a constant-one row against [bv | 128.0], which also makes
  bias handling exact: (numer + bv*denom)/denom = attn_out + bv.
"""

from contextlib import ExitStack

import numpy as np
import ml_dtypes

import concourse.bass as bass
import concourse.tile as tile
from concourse import bacc
from concourse import mybir

F32 = mybir.dt.float32
F32R = mybir.dt.float32r
BF16 = mybir.dt.bfloat16
EXP = mybir.ActivationFunctionType.Exp

B, S, D, NH, DK, DV = 2, 2048, 1024, 16, 64, 64
NCORES = 8
HPC = NH // NCORES  # heads per core == 2


def build_nc(b=B, s=S, d=D, dk=DK):
    """Build the per-core Bass program (identical on all 8 cores)."""
    nc = bacc.Bacc("TRN2", target_bir_lowering=False, debug=False)

    sq_t = min(512, s)          # sq tile (matmul free dim)
    n_sq = s // sq_t
    n_sk = s // 128             # sk tiles of 128
    n_ch = d // 128             # contraction chunks of 128
    n_dh = (d + 511) // 512     # output d halves
    d_h = min(512, d)

    qT_d = nc.dram_tensor("qT", [b, d, s], BF16, kind="ExternalInput").ap()
    kT_d = nc.dram_tensor("kT", [b, d, s], BF16, kind="ExternalInput").ap()
    vT_d = nc.dram_tensor("vT", [b, d, s], BF16, kind="ExternalInput").ap()
    wq_d = nc.dram_tensor("wq", [d, 128], BF16, kind="ExternalInput").ap()
    wk_d = nc.dram_tensor("wk", [d, 128], BF16, kind="ExternalInput").ap()
    wv_d = nc.dram_tensor("wv", [d, 130], BF16, kind="ExternalInput").ap()
    bqk_d = nc.dram_tensor("bqk", [128, 2], F32, kind="ExternalInput").ap()
    bvr_d = nc.dram_tensor("bv_row", [1, 130], BF16, kind="ExternalInput").ap()
    woa_d = nc.dram_tensor("wo_a", [64, d], BF16, kind="ExternalInput").ap()
    ones16_d = nc.dram_tensor("ones16", [1, 128], BF16, kind="ExternalInput").ap()
    onesr_d = nc.dram_tensor("onesr", [128, 64], F32R, kind="ExternalInput").ap()
    wob_d = nc.dram_tensor("wo_b", [64, d], BF16, kind="ExternalInput").ap()
    out_d = nc.dram_tensor("out", [b, s, d], F32, kind="ExternalOutput").ap()

    with tile.TileContext(nc) as tc, ExitStack() as ctx:
        consts = ctx.enter_context(tc.tile_pool(name="consts", bufs=1))
        qk_stream = ctx.enter_context(tc.tile_pool(name="qk_stream", bufs=8))
        vt_stream = ctx.enter_context(tc.tile_pool(name="vt_stream", bufs=3))
        qkt_pool = ctx.enter_context(tc.tile_pool(name="qkt", bufs=2))
        vh_pool = ctx.enter_context(tc.tile_pool(name="vh", bufs=2))
        exp_pool = ctx.enter_context(tc.tile_pool(name="expp", bufs=4))
        cat_pool = ctx.enter_context(tc.tile_pool(name="cat", bufs=2))
        recip_pool = ctx.enter_context(tc.tile_pool(name="recip", bufs=2))
        out_pool = ctx.enter_context(tc.tile_pool(name="outp", bufs=3))
        ps = ctx.enter_context(tc.tile_pool(name="ps", bufs=4, space="PSUM"))

        # --- constants ---
        wq_sb = consts.tile([128, n_ch, 128], BF16, tag="wq")
        wk_sb = consts.tile([128, n_ch, 128], BF16, tag="wk")
        wv_sb = consts.tile([128, n_ch, 130], BF16, tag="wv")
        for c in range(n_ch):
            nc.sync.dma_start(wq_sb[:, c, :], wq_d[c * 128:(c + 1) * 128, :])
            nc.sync.dma_start(wk_sb[:, c, :], wk_d[c * 128:(c + 1) * 128, :])
            nc.sync.dma_start(wv_sb[:, c, :], wv_d[c * 128:(c + 1) * 128, :])
        bqk_sb = consts.tile([128, 2], F32, tag="bqk")
        nc.sync.dma_start(bqk_sb[:], bqk_d[:])
        bvr_sb = consts.tile([1, 130], BF16, tag="bvr")
        nc.sync.dma_start(bvr_sb[:], bvr_d[:])
        woa_sb = consts.tile([64, d], BF16, tag="woa")
        nc.sync.dma_start(woa_sb[:], woa_d[:])
        wob_sb = consts.tile([64, d], BF16, tag="wob")
        nc.sync.dma_start(wob_sb[:], wob_d[:])
        ones_bf = consts.tile([1, 128], BF16, tag="ones_bf")
        nc.sync.dma_start(ones_bf[:], ones16_d[:])
        ones_fr = consts.tile([128, 64], F32R, tag="ones_fr")
        nc.sync.dma_start(ones_fr[:], onesr_d[:])

        for bi in range(b):
            # ---- Phase P1: q/k head projections -> qhT/khT [128, s] (f32r)
            qhT = qkt_pool.tile([128, s], BF16, tag="qhT")
            khT = qkt_pool.tile([128, s], BF16, tag="khT")
            qt_cs, kt_cs = [], []
            for c in range(n_ch):
                qt_c = qk_stream.tile([128, s], BF16, tag="qt", name=f"qt{c}")
                nc.sync.dma_start(qt_c[:], qT_d[bi, c * 128:(c + 1) * 128, :])
                qt_cs.append(qt_c)
                kt_c = qk_stream.tile([128, s], BF16, tag="kt", name=f"kt{c}")
                nc.sync.dma_start(kt_c[:], kT_d[bi, c * 128:(c + 1) * 128, :])
                kt_cs.append(kt_c)
            # two s-tile halves so q+k projections fit in 4 psum banks
            for half in range(0, n_sq, 2):
                sts = [st for st in (half, half + 1) if st < n_sq]
                pq = {st: ps.tile([128, sq_t], F32, tag="ps", name=f"psq{st}")
                      for st in sts}
                pk = {st: ps.tile([128, sq_t], F32, tag="ps", name=f"psk{st}")
                      for st in sts}
                for c in range(n_ch):
                    for st in sts:
                        ssl = bass.ts(st, sq_t)
                        nc.tensor.matmul(pq[st][:], wq_sb[:, c, :],
                                         qt_cs[c][:, ssl],
                                         start=(c == 0), stop=(c == n_ch - 1))
                        nc.tensor.matmul(pk[st][:], wk_sb[:, c, :],
                                         kt_cs[c][:, ssl],
                                         start=(c == 0), stop=(c == n_ch - 1))
                for st in sts:
                    ssl = bass.ts(st, sq_t)
                    with nc.allow_low_precision(reason="f32r == f32 bits"):
                        nc.vector.tensor_scalar_add(qhT[:, ssl], pq[st][:],
                                                    bqk_sb[:, 0:1])
                        nc.vector.tensor_scalar_add(khT[:, ssl], pk[st][:],
                                                    bqk_sb[:, 1:2])

            # ---- Phase P2: v projection -> vh [128(t), n_sk, 130] (bf16)
            # per t-tile psum cols: [vhA+bvA (64) | 128.0 | vhB+bvB (64) | 128.0]
            vh = vh_pool.tile([128, n_sk, 130], BF16, tag="vh")
            for tt in range(n_sk):
                vt_c = vt_stream.tile([128, n_ch, 128], BF16, tag="vt")
                nc.sync.dma_start(
                    vt_c[:],
                    vT_d[bi].rearrange("(c p) s -> p c s", p=128)[
                        :, :, bass.ts(tt, 128)],
                )
                psv = ps.tile([128, 512], F32, tag="ps")
                for c in range(n_ch):
                    nc.tensor.matmul(psv[:, 0:130], vt_c[:, c, :], wv_sb[:, c, :],
                                     start=(c == 0), stop=False)
                nc.tensor.matmul(psv[:, 0:130], ones_bf[:], bvr_sb[:],
                                 start=False, stop=True)
                nc.any.tensor_copy(vh[:, tt, 0:65], psv[:, 0:65])
                nc.any.tensor_copy(vh[:, tt, 65:130], psv[:, 65:130])

            # ---- Phase A: attention for both heads
            catA = cat_pool.tile([64, s], BF16, tag="catA")
            catB = cat_pool.tile([64, s], BF16, tag="catB")
            pending_norm = None
            for sq in range(n_sq):
                ssl = bass.ts(sq, sq_t)
                nA = ps.tile([128, sq_t], F32, tag="ps")
                nB = ps.tile([128, sq_t], F32, tag="ps")
                for k in range(n_sk):
                    ksl = bass.ts(k, 128)
                    # both heads' scoresT into one 2-bank psum -> one exp
                    sAB = ps.tile([128, 2 * sq_t], F32, tag="ps2", bufs=2)
                    nc.tensor.matmul(sAB[:, 0:sq_t], khT[0:64, ksl],
                                     qhT[0:64, ssl],
                                     start=True, stop=True, tile_position=(0, 0))
                    nc.tensor.matmul(sAB[:, sq_t:2 * sq_t], khT[64:128, ksl],
                                     qhT[64:128, ssl],
                                     start=True, stop=True, tile_position=(64, 0))
                    eAB = exp_pool.tile([128, 2 * sq_t], BF16, tag="eAB")
                    nc.scalar.activation(eAB[:], sAB[:], EXP)
                    nc.tensor.matmul(nA[0:65, :], vh[:, k, 0:65], eAB[:, 0:sq_t],
                                     start=(k == 0), stop=(k == n_sk - 1))
                    nc.tensor.matmul(nB[0:65, :], vh[:, k, 65:130],
                                     eAB[:, sq_t:2 * sq_t],
                                     start=(k == 0), stop=(k == n_sk - 1))
                # free nA/nB quickly: copy numerators + denominators out of
                # PSUM before the slow reciprocal runs.
                numAB = recip_pool.tile([64, 2 * sq_t], F32, tag="numAB")
                nc.any.tensor_copy(numAB[:, 0:sq_t], nA[0:64, :])
                nc.any.tensor_copy(numAB[:, sq_t:2 * sq_t], nB[0:64, :])
                rec = recip_pool.tile([128, 4 * sq_t], F32R, tag="rec")
                nc.vector.tensor_copy(rec[64:65, 0:sq_t], nA[64:65, :])
                nc.vector.tensor_copy(rec[64:65, sq_t:2 * sq_t], nB[64:65, :])
                with nc.allow_low_precision(reason="f32r == f32 bits"):
                    nc.vector.reciprocal(rec[64:65, 2 * sq_t:3 * sq_t],
                                         rec[64:65, 0:sq_t])
                    nc.vector.reciprocal(rec[64:65, 3 * sq_t:4 * sq_t],
                                         rec[64:65, sq_t:2 * sq_t])
                def _normalize(ssl=ssl, rec=rec, numAB=numAB):
                    # deferred one sq-tile so the reciprocal latency hides
                    # under the next k-loop instead of stalling the PE queue
                    bcA = ps.tile([128, sq_t], F32, tag="ps", name="bcA")
                    bcB = ps.tile([128, sq_t], F32, tag="ps", name="bcB")
                    nc.tensor.matmul(bcA[0:64, :], ones_fr[64:65, :],
                                     rec[64:65, 2 * sq_t:3 * sq_t],
                                     start=True, stop=True)
                    nc.tensor.matmul(bcB[0:64, :], ones_fr[64:65, :],
                                     rec[64:65, 3 * sq_t:4 * sq_t],
                                     start=True, stop=True)
                    nc.vector.tensor_mul(catA[:, ssl], bcA[0:64, :],
                                         numAB[:, 0:sq_t])
                    nc.vector.tensor_mul(catB[:, ssl], bcB[0:64, :],
                                         numAB[:, sq_t:2 * sq_t])
                if pending_norm is not None:
                    pending_norm()
                pending_norm = _normalize

            if pending_norm is not None:
                pending_norm()
                pending_norm = None

            # ---- Phase O: output projection (per-head K=64 accumulate)
            for ot in range(s // 128):
                osl = bass.ts(ot, 128)
                o_sb = out_pool.tile([128, d], F32, tag="o")
                for dh in range(n_dh):
                    dsl = bass.ts(dh, d_h)
                    po = ps.tile([128, d_h], F32, tag="ps")
                    nc.tensor.matmul(po[:], catA[:, osl], woa_sb[:, dsl],
                                     start=True, stop=False)
                    nc.tensor.matmul(po[:], catB[:, osl], wob_sb[:, dsl],
                                     start=False, stop=True)
                    nc.any.tensor_copy(o_sb[:, dsl], po[:])
                nc.sync.dma_start(out_d[bi, ot * 128:(ot + 1) * 128, :], o_sb[:])

    nc.compile()
    return nc


def make_core_inputs(Q, K, V, Wq, bq, Wk, bk, Wv, bv, Wo):
    """Host-side prep: transposes, casts, per-core weight packing."""
    bf = ml_dtypes.bfloat16
    QT = np.ascontiguousarray(
        np.transpose(np.asarray(Q, np.float32), (0, 2, 1))).astype(bf)
    KT = np.ascontiguousarray(
        np.transpose(np.asarray(K, np.float32), (0, 2, 1))).astype(bf)
    VT = np.ascontiguousarray(
        np.transpose(np.asarray(V, np.float32), (0, 2, 1))).astype(bf)
    d = QT.shape[1]

    in_maps = []
    for c in range(NCORES):
        hA, hB = HPC * c, HPC * c + 1
        wq = np.concatenate([Wq[hA], Wq[hB]], axis=1).astype(np.float32).astype(bf)
        wk = np.concatenate([Wk[hA], Wk[hB]], axis=1).astype(np.float32).astype(bf)
        wv = np.zeros((d, 130), np.float32)
        wv[:, 0:64] = Wv[hA]
        wv[:, 65:129] = Wv[hB]
        bvr = np.zeros((1, 130), np.float32)
        bvr[0, 0:64] = bv[hA]
        bvr[0, 64] = 128.0
        bvr[0, 65:129] = bv[hB]
        bvr[0, 129] = 128.0
        bqk = np.stack(
            [np.concatenate([bq[hA], bq[hB]]), np.concatenate([bk[hA], bk[hB]])],
            axis=1).astype(np.float32)
        in_maps.append({
            "qT": QT, "kT": KT, "vT": VT,
            "wq": wq, "wk": wk, "wv": wv.astype(bf),
            "bqk": bqk, "bv_row": bvr.astype(bf),
            "ones16": np.ones((1, 128), np.float32).astype(bf),
            "onesr": np.ones((128, 64), np.float32),
            "wo_a": np.asarray(Wo[64 * hA:64 * hA + 64], np.float32).astype(bf),
            "wo_b": np.asarray(Wo[64 * hB:64 * hB + 64], np.float32).astype(bf),
        })
    return in_maps


_NC_CACHE = {}


def _get_nc():
    if "nc" not in _NC_CACHE:
        _NC_CACHE["nc"] = build_nc()
    return _NC_CACHE["nc"]


def _install_ntff_hook_shim():
    """The agent image's antenv lacks axon_hooks; recreate the tiny
    get/set registry and register the ctypes NTFF profiler so trace=True
    can report HW exec time."""
    import sys
    import types
    if "antenv.axon_hooks" in sys.modules:
        return
    hook = None
    try:
        from trn_agent_boot.trn_boot import _ntff_profile_via_ctypes
        hook = _ntff_profile_via_ctypes("/opt/axon/libaxon_pjrt.so")
    except Exception:
        hook = None
    mod = types.ModuleType("antenv.axon_hooks")
    mod._hook = hook
    mod.get_axon_ntff_profile_hook = lambda: mod._hook
    mod.set_axon_ntff_profile_hook = lambda h: setattr(mod, "_hook", h)
    sys.modules["antenv.axon_hooks"] = mod


def kernel(Q, K, V, Wq, bq, Wk, bk, Wv, bv, Wo, bo, _trace=False):
    from concourse.bass_utils import run_bass_kernel_spmd

    if _trace:
        _install_ntff_hook_shim()

    nc = _get_nc()
    in_maps = make_core_inputs(Q, K, V, Wq, bq, Wk, bk, Wv, bv, Wo)
    res = run_bass_kernel_spmd(nc, in_maps, list(range(NCORES)), trace=_trace)
    out = np.zeros((B, S, D), np.float32)
    for r in res.results:
        out += np.asarray(r["out"], np.float32)
    out += np.asarray(bo, np.float32)[None, None, :]
    if _trace:
        return out, res
    return out
